# revision 14
# baseline (speedup 1.0000x reference)
"""AttentionPerLabelWordLevel Trainium2 kernel (8-core SPMD, batch-sharded).

Reference computation (per batch b):
  h = tanh(x @ W.T + b)                      # [T, H]
  logits = h @ C.T                           # [S, L, C]
  m = max_L(logits)                          # [S, 1, C]
  attn = softmax_C(logits - m)               # [S, L, C]
  out[s, c, :] = sum_l attn[s, l, c] * x[s, l, :]   # [S, C, H]

Shapes: B=32, T=2500 (S=100 sentences x L=25 words), H=512, C=50.
Sharding: data-parallel over batch, 4 batches per core, processed as one
stream of 400 sentences = 25 waves x 16 sentences (no partial waves).

Per-core pipeline (x, W, C pre-cast to f16 on host):
  - x DMA'd per wave into a packed tile [128, 4x520]: 4 groups of 4
    sentences at partition offsets 0/32/64/96 (32 rows per slot so pad
    rows hold real, finite data).
  - x^T via 16 f16 identity matmuls (M=128 -> fast weight load) into
    f32 PSUM, copied to f16 SBUF by DVE.
  - h^T = tanh(W @ x^T + b) per o-chunk; logits[c,t] accumulated over o.
  - Softmax: max over words (strided), subtract, exp -> e[c,t] f16 with
    pad columns zeroed on gpsimd.
  - e^T via 4 identity matmuls -> f32 PSUM; 4 strided DVE multiplies
    build a block-diagonal, softmax-normalized attn tile [128, 4x100]
    (normalization fused into the scatter as a broadcast multiply).
  - Output einsum: 8 matmuls per wave (2 sentences each: K=64 row-packed,
    M=100) into 2-bank f32 PSUM tiles; output DMA'd straight from PSUM
    (f32, 4 sentences per DMA, fully contiguous in DRAM) on the gpsimd
    SWDGE queue - no staging copies.
  - Software pipeline: e^T lags 1 wave, output einsum lags 2 waves and is
    sprinkled between mm1 chunks so the PE never waits on the softmax
    chain and PSUM-store drains overlap compute.
"""

import numpy as np

import concourse.bacc as bacc
import concourse.bass as bass
import concourse.tile as tile
from concourse import mybir
from concourse.bass_utils import run_bass_kernel_spmd
from concourse.masks import make_identity

F32 = mybir.dt.float32
F16 = mybir.dt.float16
AX = mybir.AxisListType
AF = mybir.ActivationFunctionType

N_CORES = 8
B = 32
S = 100          # sentences per batch
L = 25           # words per sentence
C = 50           # classes
H = 512          # hidden
B_LOC = B // N_CORES          # batches per core
NSENT = B_LOC * S             # 400 sentences per core
WAVE_S = 16                   # sentences per wave
N_W = NSENT // WAVE_S         # 25 full waves
TW = 10000                    # total words per core
WSTRIDE = 4 * L * H           # 4-sentence stride in x (elements)

_CACHE = {}
LAST_RESULT = None


def build_nc():
    nc = bacc.Bacc(trn_type="TRN2", target_bir_lowering=False, debug=False,
                   num_swdge_queues=2)
    x_d = nc.declare_dram_parameter("input_tensor", [TW, H], F16, isOutput=False)
    w_d = nc.declare_dram_parameter("W", [H, H], F16, isOutput=False)
    b_d = nc.declare_dram_parameter("b", [H], F32, isOutput=False)
    c_d = nc.declare_dram_parameter("context_vector", [C, H], F16, isOutput=False)
    o_d = nc.declare_dram_parameter("out", [NSENT * C, H], F16, isOutput=True)

    with tile.TileContext(nc) as tc:
        with tc.tile_pool(name="sb", bufs=1) as sb, \
             tc.tile_pool(name="consts", bufs=1) as consts, \
             tc.tile_pool(name="ps", bufs=1, space="PSUM") as ps:

            # ---------------- one-time setup ----------------
            ident_f = consts.tile([128, 128], F32)
            make_identity(nc, ident_f)
            ident_h = consts.tile([128, 128], F16)
            nc.vector.tensor_copy(ident_h, ident_f)

            # W^T tiles via PE transposes: w_t[i] is [i-part 128, o 512] f16
            w_nat = []
            for o in range(4):
                wn = consts.tile([128, 512], F16, name=f"w_nat{o}")
                (nc.scalar if o % 2 == 0 else nc.gpsimd).dma_start(
                    out=wn, in_=w_d[o * 128:(o + 1) * 128, :])
                w_nat.append(wn)
            c_h = consts.tile([64, 512], F16)
            nc.gpsimd.dma_start(out=c_h[:C, :], in_=c_d[:, :])
            b_sb = consts.tile([128, 4], F32)
            nc.scalar.dma_start(out=b_sb, in_=b_d.rearrange("(k p) -> p k", p=128))

            w_t = []
            for i in range(4):
                pwt = ps.tile([128, 512], F16, tag="misc", bufs=3,
                              name=f"pwt{i}")
                for o in range(4):
                    nc.tensor.transpose(
                        pwt[:, o * 128:(o + 1) * 128],
                        w_nat[o][:, i * 128:(i + 1) * 128],
                        ident_h,
                    )
                wt = consts.tile([128, 512], F16, name=f"w_t{i}")
                nc.vector.tensor_copy(wt.bitcast(F32), pwt.bitcast(F32))
                w_t.append(wt)

            # C^T tile: [o-part 128, o_chunk 4, c 64] f16
            pct = ps.tile([128, 4, 64], F16, tag="misc", bufs=3, name="pct")
            for o in range(4):
                nc.tensor.transpose(
                    pct[:, o, :C], c_h[:C, o * 128:(o + 1) * 128],
                    ident_h[:C, :C],
                )
            c_t = consts.tile([128, 4, 64], F16)
            nc.vector.tensor_copy(c_t.bitcast(F32), pct.bitcast(F32))

            # ---------------- per-wave pieces ----------------
            state = {}

            def emit_load(w):
                """DMA x for wave w into a packed tile (4 DMAs on sync)."""
                xp_all = sb.tile([128, 2088], F16, tag="xp", bufs=5,
                                 name=f"xp{w}")
                nrows = 32 if w < N_W - 1 else L
                qs = ([nc.sync, nc.sync, nc.gpsimd, nc.gpsimd] if w < 2
                      else [nc.sync] * 4)
                for jj in range(4):
                    dvw = xp_all[32 * jj:32 * jj + nrows, :]
                    dst = bass.AP(tensor=xp_all.tensor, offset=dvw.offset,
                                  ap=[dvw.ap[0], [520, 4], [1, 512]])
                    w0 = (WAVE_S * w + jj) * L
                    svw = x_d[w0:w0 + 1, :]
                    src = bass.AP(tensor=svw.tensor, offset=svw.offset,
                                  ap=[[512, nrows], [WSTRIDE, 4], [1, 512]])
                    qs[jj].dma_start(out=dst, in_=src)
                state[("xp", w)] = xp_all

            def xp_g(w, g, r0, nr):
                xp_all = state[("xp", w)]
                return xp_all[r0:r0 + nr, 520 * g:520 * g + 512]

            def emit_xt(w, i):
                """x^T i-chunk via identity matmuls (f32 psum) + DVE copy."""
                if i == 0:
                    state[("xt", w)] = sb.tile([128, 2048], F16, tag="xt",
                                               bufs=2, name=f"xt{w}")
                xt = state[("xt", w)]
                pxt = ps.tile([128, 512], F32, tag="misc", bufs=3,
                              name=f"pxt{w}_{i}")
                for g in range(4):
                    nc.tensor.matmul(
                        pxt[:, 128 * g:128 * (g + 1)],
                        xp_g(w, g, 0, 128)[:, 128 * i:128 * (i + 1)],
                        ident_h,
                        start=True, stop=True,
                    )
                if i < 2:
                    nc.vector.tensor_copy(xt[:, 512 * i:512 * (i + 1)], pxt)
                else:
                    nc.scalar.copy(xt[:, 512 * i:512 * (i + 1)], pxt)

            def emit_mm1(w, o):
                """h^T[o] = tanh(W[o,:] @ x^T + b[o])."""
                if o == 0:
                    state[("h", w)] = sb.tile([128, 2048], F16, tag="h",
                                              bufs=2, name=f"h{w}")
                h = state[("h", w)]
                xt = state[("xt", w)]
                ph = ps.tile([128, 512], F32, tag="ph", bufs=3,
                             name=f"ph{w}_{o}")
                for i in range(4):
                    nc.tensor.matmul(
                        ph, w_t[i][:, o * 128:(o + 1) * 128],
                        xt[:, 512 * i:512 * (i + 1)],
                        start=(i == 0), stop=(i == 3),
                    )
                nc.scalar.activation(
                    out=h[:, 512 * o:512 * (o + 1)], in_=ph,
                    func=AF.Tanh, bias=b_sb[:, o:o + 1], scale=1.0,
                )

            def emit_mm2(w):
                """logits[c, t] accumulated over o-chunks."""
                h = state[("h", w)]
                pl = ps.tile([64, 512], F32, tag="misc", bufs=3,
                             name=f"pl{w}")
                for o in range(4):
                    nc.tensor.matmul(
                        pl[:C, :], c_t[:, o, :C], h[:, 512 * o:512 * (o + 1)],
                        start=(o == 0), stop=(o == 3),
                    )
                state[("pl", w)] = pl

            def emit_softmax(w):
                """m = max_L, e = exp(logits - m), pad cols of e zeroed."""
                pl = state.pop(("pl", w))
                m = sb.tile([64, 16], F32, tag="m", bufs=2, name=f"m{w}")
                epre = sb.tile([64, 512], F16, tag="epre", bufs=2,
                               name=f"epre{w}")
                e_sb = sb.tile([64, 512], F16, tag="e", bufs=2,
                               name=f"e{w}")
                pv = e_sb[:C, 25:]
                pad_v = bass.AP(tensor=e_sb.tensor, offset=pv.offset,
                                ap=[pv.ap[0], [32, 16], [1, 7]])
                nc.gpsimd.memset(pad_v, 0.0)
                plv = pl[:C, :]
                pl_v = bass.AP(tensor=pl.tensor, offset=plv.offset,
                               ap=[plv.ap[0], [32, 16], [1, L]])
                nc.vector.reduce_max(out=m[:C, :], in_=pl_v, axis=AX.X)
                epv = epre[:C, :]
                ep_v = bass.AP(tensor=epre.tensor, offset=epv.offset,
                               ap=[epv.ap[0], [32, 16], [1, L]])
                ev = e_sb[:C, :]
                e_v = bass.AP(tensor=e_sb.tensor, offset=ev.offset,
                              ap=[ev.ap[0], [32, 16], [1, L]])
                mv = m[:C, :]
                m_v = bass.AP(tensor=m.tensor, offset=mv.offset,
                              ap=[mv.ap[0], [1, 16], [0, L]])
                nc.vector.tensor_sub(ep_v, pl_v, m_v)
                nc.scalar.activation(out=e_v, in_=ep_v, func=AF.Exp)
                state[("e", w)] = e_sb

            def emit_et(w):
                """e^T via 4 identity matmuls -> f32 psum (pe4)."""
                e_sb = state.pop(("e", w))
                pe4 = ps.tile([128, 4, 64], F32, tag="misc", bufs=3,
                              name=f"pe4_{w}")
                for g in range(4):
                    nc.tensor.matmul(
                        pe4[:, g, :C],
                        e_sb[:C, 128 * g:128 * (g + 1)],
                        ident_h[:C, :C],
                        start=True, stop=True,
                    )
                state[("pe4", w)] = pe4

            def emit_norm(w):
                """Block-diagonal attn tile with fused softmax-C normalize."""
                pe4 = state.pop(("pe4", w))
                attn = sb.tile([128, 4, 100], F16, tag="attn", bufs=3,
                               name=f"attn{w}")
                nc.gpsimd.memset(attn, 0.0)
                z = sb.tile([128, 4], F32, tag="z", bufs=2, name=f"z{w}")
                sv = pe4[:, :, :C]
                sum_v = bass.AP(tensor=pe4.tensor, offset=sv.offset,
                                ap=[sv.ap[0], [64, 4], [1, C]])
                nc.vector.reduce_sum(out=z, in_=sum_v, axis=AX.X)
                nc.vector.tensor_scalar_max(z, z, 1e-30)
                nc.vector.reciprocal(out=z, in_=z)
                # scatter+normalize: row block rb -> cols 0-49 (rb even) or
                # 50-99 (rb odd); multiply by z broadcast along C
                for rb in range(4):
                    r0 = 32 * rb
                    co = 50 * (rb % 2)
                    sv = pe4[r0:r0 + 32, :, :C]
                    src = bass.AP(tensor=pe4.tensor, offset=sv.offset,
                                  ap=[sv.ap[0], [64, 4], [1, C]])
                    zv = z[r0:r0 + 32, :]
                    z_v = bass.AP(tensor=z.tensor, offset=zv.offset,
                                  ap=[zv.ap[0], [1, 4], [0, C]])
                    dv = attn[r0:r0 + 32, :, co:co + C]
                    dst = bass.AP(tensor=attn.tensor, offset=dv.offset,
                                  ap=[dv.ap[0], [100, 4], [1, C]])
                    nc.vector.tensor_mul(dst, src, z_v)
                state[("attn", w)] = attn

            def emit_step5(w, t):
                """Output pair t (t=0..3): 4 sentences = 2 matmuls into a
                2-bank f32 psum tile, one cast-copy to f16 staging, and a
                DMA per 8 sentences."""
                g = t
                attn = state[("attn", w)]
                po = ps.tile([128, 1024], F32, tag="po", bufs=1,
                             name=f"po{w}_{t}")
                for half in range(2):
                    nc.tensor.matmul(
                        po[:100, 512 * half:512 * (half + 1)],
                        attn[64 * half:64 * half + 64, g, :],
                        xp_g(w, g, 64 * half, 64),
                        start=True, stop=True,
                        tile_position=(64 * half, 0),
                    )
                ti = t // 2
                if t % 2 == 0:
                    state[("osb", w, ti)] = sb.tile(
                        [128, 4, 512], F16, tag="osb", bufs=4,
                        name=f"osb{w}_{ti}")
                osb = state[("osb", w, ti)]
                dchunk = osb[:100, 2 * (t % 2):2 * (t % 2) + 2, :]
                if t % 2 == 0:
                    nc.scalar.copy(dchunk, po[:100, :])
                else:
                    nc.vector.tensor_copy(dchunk, po[:100, :])
                if t % 2 == 1:
                    osb = state.pop(("osb", w, ti))
                    src = osb[0:100, :, :]
                    row0 = (WAVE_S * w + 8 * ti) * C
                    dvw = o_d[row0:row0 + 1, :]
                    dst = bass.AP(tensor=dvw.tensor, offset=dvw.offset,
                                  ap=[[512, 100], [2 * C * 512, 4], [1, 512]])
                    nc.gpsimd.dma_start(out=dst, in_=src)

            # ---------------- pipelined main loop ----------------
            emit_load(0)
            emit_load(1)
            for i in range(4):
                emit_xt(0, i)

            for it in range(N_W):
                if it + 2 < N_W:
                    emit_load(it + 2)
                for o in range(4):
                    if it + 1 < N_W:
                        emit_xt(it + 1, o)
                    emit_mm1(it, o)
                    if it >= 2:
                        emit_step5(it - 2, o)
                emit_mm2(it)
                emit_softmax(it)
                if it >= 1:
                    emit_et(it - 1)
                    emit_norm(it - 1)
                if it == N_W - 1:
                    emit_et(it)
                    emit_norm(it)

            # epilogue: drain with 1-bank psum pieces (ph tag is free)
            def emit_step5_drain(w, t):
                g = t
                attn = state[("attn", w)]
                ti = t // 2
                if t % 2 == 0:
                    state[("osb", w, ti)] = sb.tile(
                        [128, 4, 512], F16, tag="osb", bufs=4,
                        name=f"osbd{w}_{ti}")
                osb = state[("osb", w, ti)]
                for half in range(2):
                    po = ps.tile([128, 512], F32, tag="ph", bufs=3,
                                 name=f"pod{w}_{t}_{half}")
                    nc.tensor.matmul(
                        po[:100, :],
                        attn[64 * half:64 * half + 64, g, :],
                        xp_g(w, g, 64 * half, 64),
                        start=True, stop=True,
                        tile_position=(64 * half, 0),
                    )
                    dchunk = osb[:100, 2 * (t % 2) + half, :]
                    if half == 0:
                        nc.scalar.copy(dchunk, po[:100, :])
                    else:
                        nc.vector.tensor_copy(dchunk, po[:100, :])
                if t % 2 == 1:
                    osb = state.pop(("osb", w, ti))
                    src = osb[0:100, :, :]
                    row0 = (WAVE_S * w + 8 * ti) * C
                    dvw = o_d[row0:row0 + 1, :]
                    dst = bass.AP(tensor=dvw.tensor, offset=dvw.offset,
                                  ap=[[512, 100], [2 * C * 512, 4], [1, 512]])
                    nc.gpsimd.dma_start(out=dst, in_=src)

            for t in range(4):
                emit_step5_drain(N_W - 2, t)
            for t in range(4):
                emit_step5_drain(N_W - 1, t)

    nc.compile()
    return nc


def kernel(**inputs):
    global LAST_RESULT
    if "nc" not in _CACHE:
        _CACHE["nc"] = build_nc()
    nc = _CACHE["nc"]

    x = np.asarray(inputs["input_tensor"], dtype=np.float32).astype(np.float16)
    w = np.asarray(inputs["W"], dtype=np.float32).astype(np.float16)
    bb = np.ascontiguousarray(inputs["b"], dtype=np.float32)
    cv = np.asarray(inputs["context_vector"], dtype=np.float32).astype(np.float16)

    in_maps = [
        {
            "input_tensor": np.ascontiguousarray(
                x[ci * B_LOC:(ci + 1) * B_LOC]).reshape(TW, H),
            "W": w,
            "b": bb,
            "context_vector": cv,
        }
        for ci in range(N_CORES)
    ]
    res = run_bass_kernel_spmd(nc, in_maps, core_ids=list(range(N_CORES)))
    LAST_RESULT = res
    out = np.empty((B, S, C, H), dtype=np.float32)
    for ci in range(N_CORES):
        out[ci * B_LOC:(ci + 1) * B_LOC] = (
            res.results[ci]["out"].astype(np.float32)
            .reshape(B_LOC, S, C, H))
    return out


# revision 15
# speedup vs baseline: 1.0725x; 1.0725x over previous
"""AttentionPerLabelWordLevel Trainium2 kernel (8-core SPMD, batch-sharded).

Reference computation (per batch b):
  h = tanh(x @ W.T + b)                      # [T, H]
  logits = h @ C.T                           # [S, L, C]
  m = max_L(logits)                          # [S, 1, C]
  attn = softmax_C(logits - m)               # [S, L, C]
  out[s, c, :] = sum_l attn[s, l, c] * x[s, l, :]   # [S, C, H]

Shapes: B=32, T=2500 (S=100 sentences x L=25 words), H=512, C=50.
Sharding: data-parallel over batch, 4 batches per core, processed as one
stream of 400 sentences = 25 waves x 16 sentences (no partial waves).

Per-core pipeline (x, W, C pre-cast to f16 on host):
  - x DMA'd per wave into a packed tile [128, 4x520]: 4 groups of 4
    sentences at partition offsets 0/32/64/96 (32 rows per slot so pad
    rows hold real, finite data).
  - x^T via 16 f16 identity matmuls (M=128 -> fast weight load) into
    f32 PSUM, copied to f16 SBUF by DVE.
  - h^T = tanh(W @ x^T + b) per o-chunk; logits[c,t] accumulated over o.
  - Softmax: max over words (strided), subtract, exp -> e[c,t] f16 with
    pad columns zeroed on gpsimd.
  - e^T via 4 identity matmuls -> f32 PSUM; 4 strided DVE multiplies
    build a block-diagonal, softmax-normalized attn tile [128, 4x100]
    (normalization fused into the scatter as a broadcast multiply).
  - Output einsum: 8 matmuls per wave (2 sentences each: K=64 row-packed,
    M=100) into 2-bank f32 PSUM tiles; output DMA'd straight from PSUM
    (f32, 4 sentences per DMA, fully contiguous in DRAM) on the gpsimd
    SWDGE queue - no staging copies.
  - Software pipeline: e^T lags 1 wave, output einsum lags 2 waves and is
    sprinkled between mm1 chunks so the PE never waits on the softmax
    chain and PSUM-store drains overlap compute.
"""

import numpy as np

import concourse.bacc as bacc
import concourse.bass as bass
import concourse.tile as tile
from concourse import mybir
from concourse.bass_utils import run_bass_kernel_spmd
from concourse.masks import make_identity

F32 = mybir.dt.float32
F16 = mybir.dt.float16
AX = mybir.AxisListType
AF = mybir.ActivationFunctionType

N_CORES = 8
B = 32
S = 100          # sentences per batch
L = 25           # words per sentence
C = 50           # classes
H = 512          # hidden
B_LOC = B // N_CORES          # batches per core
NSENT = B_LOC * S             # 400 sentences per core
WAVE_S = 16                   # sentences per wave
N_W = NSENT // WAVE_S         # 25 full waves
TW = 10000                    # total words per core
WSTRIDE = 4 * L * H           # 4-sentence stride in x (elements)

_CACHE = {}
LAST_RESULT = None


def build_nc():
    nc = bacc.Bacc(trn_type="TRN2", target_bir_lowering=False, debug=False,
                   num_swdge_queues=2)
    x_d = nc.declare_dram_parameter("input_tensor", [TW, H], F16, isOutput=False)
    w_d = nc.declare_dram_parameter("W", [H, H], F16, isOutput=False)
    b_d = nc.declare_dram_parameter("b", [H], F32, isOutput=False)
    c_d = nc.declare_dram_parameter("context_vector", [C, H], F16, isOutput=False)
    o_d = nc.declare_dram_parameter("out", [NSENT * C, H], F16, isOutput=True)

    with tile.TileContext(nc) as tc:
        with tc.tile_pool(name="sb", bufs=1) as sb, \
             tc.tile_pool(name="consts", bufs=1) as consts, \
             tc.tile_pool(name="ps", bufs=1, space="PSUM") as ps:

            # ---------------- one-time setup ----------------
            ident_f = consts.tile([128, 128], F32)
            make_identity(nc, ident_f)
            ident_h = consts.tile([128, 128], F16)
            nc.vector.tensor_copy(ident_h, ident_f)

            # W^T tiles via PE transposes: w_t[i] is [i-part 128, o 512] f16
            w_nat = []
            for o in range(4):
                wn = consts.tile([128, 512], F16, name=f"w_nat{o}")
                (nc.scalar if o % 2 == 0 else nc.gpsimd).dma_start(
                    out=wn, in_=w_d[o * 128:(o + 1) * 128, :])
                w_nat.append(wn)
            c_h = consts.tile([64, 512], F16)
            nc.gpsimd.dma_start(out=c_h[:C, :], in_=c_d[:, :])
            b_sb = consts.tile([128, 4], F32)
            nc.scalar.dma_start(out=b_sb, in_=b_d.rearrange("(k p) -> p k", p=128))

            w_t = []
            for i in range(4):
                pwt = ps.tile([128, 512], F16, tag="misc", bufs=3,
                              name=f"pwt{i}")
                for o in range(4):
                    nc.tensor.transpose(
                        pwt[:, o * 128:(o + 1) * 128],
                        w_nat[o][:, i * 128:(i + 1) * 128],
                        ident_h,
                    )
                wt = consts.tile([128, 512], F16, name=f"w_t{i}")
                nc.vector.tensor_copy(wt.bitcast(F32), pwt.bitcast(F32))
                w_t.append(wt)

            # C^T tile: [o-part 128, o_chunk 4, c 64] f16
            pct = ps.tile([128, 4, 64], F16, tag="misc", bufs=3, name="pct")
            for o in range(4):
                nc.tensor.transpose(
                    pct[:, o, :C], c_h[:C, o * 128:(o + 1) * 128],
                    ident_h[:C, :C],
                )
            c_t = consts.tile([128, 4, 64], F16)
            nc.vector.tensor_copy(c_t.bitcast(F32), pct.bitcast(F32))

            # ---------------- per-wave pieces ----------------
            state = {}

            def emit_load(w):
                """DMA x for wave w into a packed tile (4 DMAs on sync)."""
                xp_all = sb.tile([128, 2088], F16, tag="xp", bufs=5,
                                 name=f"xp{w}")
                nrows = 32 if w < N_W - 1 else L
                qs = ([nc.sync, nc.sync, nc.gpsimd, nc.gpsimd] if w < 2
                      else [nc.sync] * 4)
                for jj in range(4):
                    dvw = xp_all[32 * jj:32 * jj + nrows, :]
                    dst = bass.AP(tensor=xp_all.tensor, offset=dvw.offset,
                                  ap=[dvw.ap[0], [520, 4], [1, 512]])
                    w0 = (WAVE_S * w + jj) * L
                    svw = x_d[w0:w0 + 1, :]
                    src = bass.AP(tensor=svw.tensor, offset=svw.offset,
                                  ap=[[512, nrows], [WSTRIDE, 4], [1, 512]])
                    qs[jj].dma_start(out=dst, in_=src)
                state[("xp", w)] = xp_all

            def xp_g(w, g, r0, nr):
                xp_all = state[("xp", w)]
                return xp_all[r0:r0 + nr, 520 * g:520 * g + 512]

            def emit_xt(w, i):
                """x^T i-chunk via identity matmuls (f32 psum) + DVE copy."""
                if i == 0:
                    state[("xt", w)] = sb.tile([128, 2048], F16, tag="xt",
                                               bufs=2, name=f"xt{w}")
                xt = state[("xt", w)]
                pxt = ps.tile([128, 512], F32, tag="misc", bufs=3,
                              name=f"pxt{w}_{i}")
                for g in range(4):
                    nc.tensor.matmul(
                        pxt[:, 128 * g:128 * (g + 1)],
                        xp_g(w, g, 0, 128)[:, 128 * i:128 * (i + 1)],
                        ident_h,
                        start=True, stop=True,
                    )
                if i < 2:
                    nc.vector.tensor_copy(xt[:, 512 * i:512 * (i + 1)], pxt)
                else:
                    nc.scalar.copy(xt[:, 512 * i:512 * (i + 1)], pxt)

            def emit_mm1(w, o):
                """h^T[o] = tanh(W[o,:] @ x^T + b[o])."""
                if o == 0:
                    state[("h", w)] = sb.tile([128, 2048], F16, tag="h",
                                              bufs=2, name=f"h{w}")
                h = state[("h", w)]
                xt = state[("xt", w)]
                ph = ps.tile([128, 512], F32, tag="ph", bufs=3,
                             name=f"ph{w}_{o}")
                for i in range(4):
                    nc.tensor.matmul(
                        ph, w_t[i][:, o * 128:(o + 1) * 128],
                        xt[:, 512 * i:512 * (i + 1)],
                        start=(i == 0), stop=(i == 3),
                    )
                nc.scalar.activation(
                    out=h[:, 512 * o:512 * (o + 1)], in_=ph,
                    func=AF.Tanh, bias=b_sb[:, o:o + 1], scale=1.0,
                )

            def emit_mm2(w):
                """logits[c, t] accumulated over o-chunks."""
                h = state[("h", w)]
                pl = ps.tile([64, 512], F32, tag="misc", bufs=3,
                             name=f"pl{w}")
                for o in range(4):
                    nc.tensor.matmul(
                        pl[:C, :], c_t[:, o, :C], h[:, 512 * o:512 * (o + 1)],
                        start=(o == 0), stop=(o == 3),
                    )
                state[("pl", w)] = pl

            def emit_softmax(w):
                """m = max_L, e = exp(logits - m), pad cols of e zeroed."""
                pl = state.pop(("pl", w))
                m = sb.tile([64, 16], F32, tag="m", bufs=2, name=f"m{w}")
                epre = sb.tile([64, 512], F16, tag="epre", bufs=2,
                               name=f"epre{w}")
                e_sb = sb.tile([64, 512], F16, tag="e", bufs=2,
                               name=f"e{w}")
                pv = e_sb[:C, 25:]
                pad_v = bass.AP(tensor=e_sb.tensor, offset=pv.offset,
                                ap=[pv.ap[0], [32, 16], [1, 7]])
                nc.gpsimd.memset(pad_v, 0.0)
                plv = pl[:C, :]
                pl_v = bass.AP(tensor=pl.tensor, offset=plv.offset,
                               ap=[plv.ap[0], [32, 16], [1, L]])
                nc.vector.reduce_max(out=m[:C, :], in_=pl_v, axis=AX.X)
                epv = epre[:C, :]
                ep_v = bass.AP(tensor=epre.tensor, offset=epv.offset,
                               ap=[epv.ap[0], [32, 16], [1, L]])
                ev = e_sb[:C, :]
                e_v = bass.AP(tensor=e_sb.tensor, offset=ev.offset,
                              ap=[ev.ap[0], [32, 16], [1, L]])
                mv = m[:C, :]
                m_v = bass.AP(tensor=m.tensor, offset=mv.offset,
                              ap=[mv.ap[0], [1, 16], [0, L]])
                nc.vector.tensor_sub(ep_v, pl_v, m_v)
                nc.scalar.activation(out=e_v, in_=ep_v, func=AF.Exp)
                state[("e", w)] = e_sb

            def emit_et(w):
                """e^T via 4 identity matmuls -> f32 psum (pe4)."""
                e_sb = state.pop(("e", w))
                pe4 = ps.tile([128, 4, 64], F32, tag="misc", bufs=3,
                              name=f"pe4_{w}")
                for g in range(4):
                    nc.tensor.matmul(
                        pe4[:, g, :C],
                        e_sb[:C, 128 * g:128 * (g + 1)],
                        ident_h[:C, :C],
                        start=True, stop=True,
                    )
                state[("pe4", w)] = pe4

            def emit_norm(w):
                """Block-diagonal attn tile with fused softmax-C normalize."""
                pe4 = state.pop(("pe4", w))
                attn = sb.tile([128, 4, 100], F16, tag="attn", bufs=3,
                               name=f"attn{w}")
                nc.gpsimd.memset(attn, 0.0)
                z = sb.tile([128, 4], F32, tag="z", bufs=2, name=f"z{w}")
                sv = pe4[:, :, :C]
                sum_v = bass.AP(tensor=pe4.tensor, offset=sv.offset,
                                ap=[sv.ap[0], [64, 4], [1, C]])
                nc.vector.reduce_sum(out=z, in_=sum_v, axis=AX.X)
                nc.vector.tensor_scalar_max(z, z, 1e-30)
                nc.vector.reciprocal(out=z, in_=z)
                # scatter+normalize: row block rb -> cols 0-49 (rb even) or
                # 50-99 (rb odd); multiply by z broadcast along C
                for rb in range(4):
                    r0 = 32 * rb
                    co = 50 * (rb % 2)
                    sv = pe4[r0:r0 + 32, :, :C]
                    src = bass.AP(tensor=pe4.tensor, offset=sv.offset,
                                  ap=[sv.ap[0], [64, 4], [1, C]])
                    zv = z[r0:r0 + 32, :]
                    z_v = bass.AP(tensor=z.tensor, offset=zv.offset,
                                  ap=[zv.ap[0], [1, 4], [0, C]])
                    dv = attn[r0:r0 + 32, :, co:co + C]
                    dst = bass.AP(tensor=attn.tensor, offset=dv.offset,
                                  ap=[dv.ap[0], [100, 4], [1, C]])
                    nc.vector.tensor_mul(dst, src, z_v)
                state[("attn", w)] = attn

            def emit_step5(w, t):
                """Output pair t (t=0..3): 4 sentences = 2 matmuls into a
                2-bank f32 psum tile, one cast-copy to f16 staging, and a
                DMA per 8 sentences."""
                g = t
                attn = state[("attn", w)]
                po = ps.tile([128, 1024], F32, tag="po", bufs=1,
                             name=f"po{w}_{t}")
                for half in range(2):
                    nc.tensor.matmul(
                        po[:100, 512 * half:512 * (half + 1)],
                        attn[64 * half:64 * half + 64, g, :],
                        xp_g(w, g, 64 * half, 64),
                        start=True, stop=True,
                        tile_position=(64 * half, 0),
                    )
                ti = t // 2
                if t % 2 == 0:
                    state[("osb", w, ti)] = sb.tile(
                        [128, 4, 512], F16, tag="osb", bufs=4,
                        name=f"osb{w}_{ti}")
                osb = state[("osb", w, ti)]
                dchunk = osb[:100, 2 * (t % 2):2 * (t % 2) + 2, :]
                if t % 2 == 0:
                    nc.scalar.copy(dchunk, po[:100, :])
                else:
                    nc.vector.tensor_copy(dchunk, po[:100, :])
                if t % 2 == 1:
                    osb = state.pop(("osb", w, ti))
                    src = osb[0:100, :, :]
                    row0 = (WAVE_S * w + 8 * ti) * C
                    dvw = o_d[row0:row0 + 1, :]
                    dst = bass.AP(tensor=dvw.tensor, offset=dvw.offset,
                                  ap=[[512, 100], [2 * C * 512, 4], [1, 512]])
                    nc.gpsimd.dma_start(out=dst, in_=src)

            # ---------------- pipelined main loop ----------------
            emit_load(0)
            emit_load(1)
            for i in range(4):
                emit_xt(0, i)

            for it in range(N_W):
                if it + 2 < N_W:
                    emit_load(it + 2)
                for o in range(4):
                    if it + 1 < N_W:
                        emit_xt(it + 1, o)
                    emit_mm1(it, o)
                    if it >= 2:
                        emit_step5(it - 2, o)
                    if o == 1 and it >= 1:
                        emit_et(it - 1)
                        emit_norm(it - 1)
                emit_mm2(it)
                emit_softmax(it)
                if it == N_W - 1:
                    emit_et(it)
                    emit_norm(it)

            # epilogue: drain with 1-bank psum pieces (ph tag is free)
            def emit_step5_drain(w, t):
                g = t
                attn = state[("attn", w)]
                ti = t // 2
                if t % 2 == 0:
                    state[("osb", w, ti)] = sb.tile(
                        [128, 4, 512], F16, tag="osb", bufs=4,
                        name=f"osbd{w}_{ti}")
                osb = state[("osb", w, ti)]
                for half in range(2):
                    po = ps.tile([128, 512], F32, tag="ph", bufs=3,
                                 name=f"pod{w}_{t}_{half}")
                    nc.tensor.matmul(
                        po[:100, :],
                        attn[64 * half:64 * half + 64, g, :],
                        xp_g(w, g, 64 * half, 64),
                        start=True, stop=True,
                        tile_position=(64 * half, 0),
                    )
                    dchunk = osb[:100, 2 * (t % 2) + half, :]
                    if half == 0:
                        nc.scalar.copy(dchunk, po[:100, :])
                    else:
                        nc.vector.tensor_copy(dchunk, po[:100, :])
                if t % 2 == 1:
                    osb = state.pop(("osb", w, ti))
                    src = osb[0:100, :, :]
                    row0 = (WAVE_S * w + 8 * ti) * C
                    dvw = o_d[row0:row0 + 1, :]
                    dst = bass.AP(tensor=dvw.tensor, offset=dvw.offset,
                                  ap=[[512, 100], [2 * C * 512, 4], [1, 512]])
                    nc.gpsimd.dma_start(out=dst, in_=src)

            for t in range(4):
                emit_step5_drain(N_W - 2, t)
            for t in range(4):
                emit_step5_drain(N_W - 1, t)

    nc.compile()
    return nc


def kernel(**inputs):
    global LAST_RESULT
    if "nc" not in _CACHE:
        _CACHE["nc"] = build_nc()
    nc = _CACHE["nc"]

    x = np.asarray(inputs["input_tensor"], dtype=np.float32).astype(np.float16)
    w = np.asarray(inputs["W"], dtype=np.float32).astype(np.float16)
    bb = np.ascontiguousarray(inputs["b"], dtype=np.float32)
    cv = np.asarray(inputs["context_vector"], dtype=np.float32).astype(np.float16)

    in_maps = [
        {
            "input_tensor": np.ascontiguousarray(
                x[ci * B_LOC:(ci + 1) * B_LOC]).reshape(TW, H),
            "W": w,
            "b": bb,
            "context_vector": cv,
        }
        for ci in range(N_CORES)
    ]
    res = run_bass_kernel_spmd(nc, in_maps, core_ids=list(range(N_CORES)))
    LAST_RESULT = res
    out = np.empty((B, S, C, H), dtype=np.float32)
    for ci in range(N_CORES):
        out[ci * B_LOC:(ci + 1) * B_LOC] = (
            res.results[ci]["out"].astype(np.float32)
            .reshape(B_LOC, S, C, H))
    return out


# revision 16
# speedup vs baseline: 1.0830x; 1.0098x over previous
"""AttentionPerLabelWordLevel Trainium2 kernel (8-core SPMD, batch-sharded).

Reference computation (per batch b):
  h = tanh(x @ W.T + b)                      # [T, H]
  logits = h @ C.T                           # [S, L, C]
  m = max_L(logits)                          # [S, 1, C]
  attn = softmax_C(logits - m)               # [S, L, C]
  out[s, c, :] = sum_l attn[s, l, c] * x[s, l, :]   # [S, C, H]

Shapes: B=32, T=2500 (S=100 sentences x L=25 words), H=512, C=50.
Sharding: data-parallel over batch, 4 batches per core, processed as one
stream of 400 sentences = 25 waves x 16 sentences (no partial waves).

Per-core pipeline (x, W, C pre-cast to f16 on host):
  - x DMA'd per wave into a packed tile [128, 4x520]: 4 groups of 4
    sentences at partition offsets 0/32/64/96 (32 rows per slot so pad
    rows hold real, finite data).
  - x^T via 16 f16 identity matmuls (M=128 -> fast weight load) into
    f32 PSUM, copied to f16 SBUF by DVE.
  - h^T = tanh(W @ x^T + b) per o-chunk; logits[c,t] accumulated over o.
  - Softmax: max over words (strided), subtract, exp -> e[c,t] f16 with
    pad columns zeroed on gpsimd.
  - e^T via 4 identity matmuls -> f32 PSUM; 4 strided DVE multiplies
    build a block-diagonal, softmax-normalized attn tile [128, 4x100]
    (normalization fused into the scatter as a broadcast multiply).
  - Output einsum: 8 matmuls per wave (2 sentences each: K=64 row-packed,
    M=100) into 2-bank f32 PSUM tiles; output DMA'd straight from PSUM
    (f32, 4 sentences per DMA, fully contiguous in DRAM) on the gpsimd
    SWDGE queue - no staging copies.
  - Software pipeline: e^T lags 1 wave, output einsum lags 2 waves and is
    sprinkled between mm1 chunks so the PE never waits on the softmax
    chain and PSUM-store drains overlap compute.
"""

import numpy as np

import concourse.bacc as bacc
import concourse.bass as bass
import concourse.tile as tile
from concourse import mybir
from concourse.bass_utils import run_bass_kernel_spmd
from concourse.masks import make_identity

F32 = mybir.dt.float32
F16 = mybir.dt.float16
AX = mybir.AxisListType
AF = mybir.ActivationFunctionType

N_CORES = 8
B = 32
S = 100          # sentences per batch
L = 25           # words per sentence
C = 50           # classes
H = 512          # hidden
B_LOC = B // N_CORES          # batches per core
NSENT = B_LOC * S             # 400 sentences per core
WAVE_S = 16                   # sentences per wave
N_W = NSENT // WAVE_S         # 25 full waves
TW = 10000                    # total words per core
WSTRIDE = 4 * L * H           # 4-sentence stride in x (elements)

_CACHE = {}
LAST_RESULT = None


def build_nc():
    nc = bacc.Bacc(trn_type="TRN2", target_bir_lowering=False, debug=False,
                   num_swdge_queues=2)
    x_d = nc.declare_dram_parameter("input_tensor", [TW, H], F16, isOutput=False)
    w_d = nc.declare_dram_parameter("W", [H, H], F16, isOutput=False)
    b_d = nc.declare_dram_parameter("b", [H], F32, isOutput=False)
    c_d = nc.declare_dram_parameter("context_vector", [C, H], F16, isOutput=False)
    o_d = nc.declare_dram_parameter("out", [NSENT * C, H], F16, isOutput=True)

    with tile.TileContext(nc) as tc:
        with tc.tile_pool(name="sb", bufs=1) as sb, \
             tc.tile_pool(name="consts", bufs=1) as consts, \
             tc.tile_pool(name="ps", bufs=1, space="PSUM") as ps:

            # ---------------- one-time setup ----------------
            ident_f = consts.tile([128, 128], F32)
            make_identity(nc, ident_f)
            ident_h = consts.tile([128, 128], F16)
            nc.vector.tensor_copy(ident_h, ident_f)

            # W^T tiles via PE transposes: w_t[i] is [i-part 128, o 512] f16
            w_nat = []
            for o in range(4):
                wn = consts.tile([128, 512], F16, name=f"w_nat{o}")
                nc.scalar.dma_start(out=wn, in_=w_d[o * 128:(o + 1) * 128, :])
                w_nat.append(wn)
            c_h = consts.tile([64, 512], F16)
            nc.gpsimd.dma_start(out=c_h[:C, :], in_=c_d[:, :])
            b_sb = consts.tile([128, 4], F32)
            nc.scalar.dma_start(out=b_sb, in_=b_d.rearrange("(k p) -> p k", p=128))

            # o-ordered so transposes start as soon as w_nat[0] lands
            pwt2 = [ps.tile([128, 1024], F16, tag="misc", bufs=3,
                            name=f"pwt2_{hh}") for hh in range(2)]
            for o in range(4):
                for i in range(4):
                    nc.tensor.transpose(
                        pwt2[i // 2][:, 512 * (i % 2) + o * 128:
                                     512 * (i % 2) + (o + 1) * 128],
                        w_nat[o][:, i * 128:(i + 1) * 128],
                        ident_h,
                    )
            w_t = []
            for i in range(4):
                wt = consts.tile([128, 512], F16, name=f"w_t{i}")
                nc.vector.tensor_copy(
                    wt.bitcast(F32),
                    pwt2[i // 2][:, 512 * (i % 2):512 * (i % 2 + 1)]
                    .bitcast(F32))
                w_t.append(wt)

            # C^T tile: [o-part 128, o_chunk 4, c 64] f16
            pct = ps.tile([128, 4, 64], F16, tag="misc", bufs=3, name="pct")
            for o in range(4):
                nc.tensor.transpose(
                    pct[:, o, :C], c_h[:C, o * 128:(o + 1) * 128],
                    ident_h[:C, :C],
                )
            c_t = consts.tile([128, 4, 64], F16)
            nc.vector.tensor_copy(c_t.bitcast(F32), pct.bitcast(F32))

            # ---------------- per-wave pieces ----------------
            state = {}

            def emit_load(w):
                """DMA x for wave w into a packed tile (4 DMAs on sync)."""
                xp_all = sb.tile([128, 2088], F16, tag="xp", bufs=5,
                                 name=f"xp{w}")
                nrows = 32 if w < N_W - 1 else L
                qs = ([nc.sync, nc.sync, nc.gpsimd, nc.gpsimd] if w < 2
                      else [nc.sync] * 4)
                for jj in range(4):
                    dvw = xp_all[32 * jj:32 * jj + nrows, :]
                    dst = bass.AP(tensor=xp_all.tensor, offset=dvw.offset,
                                  ap=[dvw.ap[0], [520, 4], [1, 512]])
                    w0 = (WAVE_S * w + jj) * L
                    svw = x_d[w0:w0 + 1, :]
                    src = bass.AP(tensor=svw.tensor, offset=svw.offset,
                                  ap=[[512, nrows], [WSTRIDE, 4], [1, 512]])
                    qs[jj].dma_start(out=dst, in_=src)
                state[("xp", w)] = xp_all

            def xp_g(w, g, r0, nr):
                xp_all = state[("xp", w)]
                return xp_all[r0:r0 + nr, 520 * g:520 * g + 512]

            def emit_xt(w, i):
                """x^T i-chunk via identity matmuls (f32 psum) + DVE copy."""
                if i == 0:
                    state[("xt", w)] = sb.tile([128, 2048], F16, tag="xt",
                                               bufs=2, name=f"xt{w}")
                xt = state[("xt", w)]
                pxt = ps.tile([128, 512], F32, tag="misc", bufs=3,
                              name=f"pxt{w}_{i}")
                for g in range(4):
                    nc.tensor.matmul(
                        pxt[:, 128 * g:128 * (g + 1)],
                        xp_g(w, g, 0, 128)[:, 128 * i:128 * (i + 1)],
                        ident_h,
                        start=True, stop=True,
                    )
                if i < 2:
                    nc.vector.tensor_copy(xt[:, 512 * i:512 * (i + 1)], pxt)
                else:
                    nc.scalar.copy(xt[:, 512 * i:512 * (i + 1)], pxt)

            def emit_mm1(w, o):
                """h^T[o] = tanh(W[o,:] @ x^T + b[o])."""
                if o == 0:
                    state[("h", w)] = sb.tile([128, 2048], F16, tag="h",
                                              bufs=2, name=f"h{w}")
                h = state[("h", w)]
                xt = state[("xt", w)]
                ph = ps.tile([128, 512], F32, tag="ph", bufs=3,
                             name=f"ph{w}_{o}")
                for i in range(4):
                    nc.tensor.matmul(
                        ph, w_t[i][:, o * 128:(o + 1) * 128],
                        xt[:, 512 * i:512 * (i + 1)],
                        start=(i == 0), stop=(i == 3),
                    )
                nc.scalar.activation(
                    out=h[:, 512 * o:512 * (o + 1)], in_=ph,
                    func=AF.Tanh, bias=b_sb[:, o:o + 1], scale=1.0,
                )

            def emit_mm2(w):
                """logits[c, t] accumulated over o-chunks."""
                h = state[("h", w)]
                pl = ps.tile([64, 512], F32, tag="misc", bufs=3,
                             name=f"pl{w}")
                for o in range(4):
                    nc.tensor.matmul(
                        pl[:C, :], c_t[:, o, :C], h[:, 512 * o:512 * (o + 1)],
                        start=(o == 0), stop=(o == 3),
                    )
                state[("pl", w)] = pl

            def emit_softmax(w):
                """m = max_L, e = exp(logits - m), pad cols of e zeroed."""
                pl = state.pop(("pl", w))
                m = sb.tile([64, 16], F32, tag="m", bufs=2, name=f"m{w}")
                epre = sb.tile([64, 512], F16, tag="epre", bufs=2,
                               name=f"epre{w}")
                e_sb = sb.tile([64, 512], F16, tag="e", bufs=2,
                               name=f"e{w}")
                pv = e_sb[:C, 25:]
                pad_v = bass.AP(tensor=e_sb.tensor, offset=pv.offset,
                                ap=[pv.ap[0], [32, 16], [1, 7]])
                nc.gpsimd.memset(pad_v, 0.0)
                plv = pl[:C, :]
                pl_v = bass.AP(tensor=pl.tensor, offset=plv.offset,
                               ap=[plv.ap[0], [32, 16], [1, L]])
                nc.vector.reduce_max(out=m[:C, :], in_=pl_v, axis=AX.X)
                epv = epre[:C, :]
                ep_v = bass.AP(tensor=epre.tensor, offset=epv.offset,
                               ap=[epv.ap[0], [32, 16], [1, L]])
                ev = e_sb[:C, :]
                e_v = bass.AP(tensor=e_sb.tensor, offset=ev.offset,
                              ap=[ev.ap[0], [32, 16], [1, L]])
                mv = m[:C, :]
                m_v = bass.AP(tensor=m.tensor, offset=mv.offset,
                              ap=[mv.ap[0], [1, 16], [0, L]])
                nc.vector.tensor_sub(ep_v, pl_v, m_v)
                nc.scalar.activation(out=e_v, in_=ep_v, func=AF.Exp)
                state[("e", w)] = e_sb

            def emit_et(w):
                """e^T via 4 identity matmuls -> f32 psum (pe4)."""
                e_sb = state.pop(("e", w))
                pe4 = ps.tile([128, 4, 64], F32, tag="misc", bufs=3,
                              name=f"pe4_{w}")
                for g in range(4):
                    nc.tensor.matmul(
                        pe4[:, g, :C],
                        e_sb[:C, 128 * g:128 * (g + 1)],
                        ident_h[:C, :C],
                        start=True, stop=True,
                    )
                state[("pe4", w)] = pe4

            def emit_norm(w):
                """Block-diagonal attn tile with fused softmax-C normalize."""
                pe4 = state.pop(("pe4", w))
                attn = sb.tile([128, 4, 100], F16, tag="attn", bufs=3,
                               name=f"attn{w}")
                nc.gpsimd.memset(attn, 0.0)
                z = sb.tile([128, 4], F32, tag="z", bufs=2, name=f"z{w}")
                sv = pe4[:, :, :C]
                sum_v = bass.AP(tensor=pe4.tensor, offset=sv.offset,
                                ap=[sv.ap[0], [64, 4], [1, C]])
                nc.vector.reduce_sum(out=z, in_=sum_v, axis=AX.X)
                nc.vector.tensor_scalar_max(z, z, 1e-30)
                nc.vector.reciprocal(out=z, in_=z)
                # scatter+normalize: row block rb -> cols 0-49 (rb even) or
                # 50-99 (rb odd); multiply by z broadcast along C
                for rb in range(4):
                    r0 = 32 * rb
                    co = 50 * (rb % 2)
                    sv = pe4[r0:r0 + 32, :, :C]
                    src = bass.AP(tensor=pe4.tensor, offset=sv.offset,
                                  ap=[sv.ap[0], [64, 4], [1, C]])
                    zv = z[r0:r0 + 32, :]
                    z_v = bass.AP(tensor=z.tensor, offset=zv.offset,
                                  ap=[zv.ap[0], [1, 4], [0, C]])
                    dv = attn[r0:r0 + 32, :, co:co + C]
                    dst = bass.AP(tensor=attn.tensor, offset=dv.offset,
                                  ap=[dv.ap[0], [100, 4], [1, C]])
                    nc.vector.tensor_mul(dst, src, z_v)
                state[("attn", w)] = attn

            def emit_step5(w, t):
                """Output pair t (t=0..3): 4 sentences = 2 matmuls into a
                2-bank f32 psum tile, one cast-copy to f16 staging, and a
                DMA per 8 sentences."""
                g = t
                attn = state[("attn", w)]
                po = ps.tile([128, 1024], F32, tag="po", bufs=1,
                             name=f"po{w}_{t}")
                for half in range(2):
                    nc.tensor.matmul(
                        po[:100, 512 * half:512 * (half + 1)],
                        attn[64 * half:64 * half + 64, g, :],
                        xp_g(w, g, 64 * half, 64),
                        start=True, stop=True,
                        tile_position=(64 * half, 0),
                    )
                ti = t // 2
                if t % 2 == 0:
                    state[("osb", w, ti)] = sb.tile(
                        [128, 4, 512], F16, tag="osb", bufs=4,
                        name=f"osb{w}_{ti}")
                osb = state[("osb", w, ti)]
                dchunk = osb[:100, 2 * (t % 2):2 * (t % 2) + 2, :]
                if t % 2 == 0:
                    nc.scalar.copy(dchunk, po[:100, :])
                else:
                    nc.vector.tensor_copy(dchunk, po[:100, :])
                if t % 2 == 1:
                    osb = state.pop(("osb", w, ti))
                    src = osb[0:100, :, :]
                    row0 = (WAVE_S * w + 8 * ti) * C
                    dvw = o_d[row0:row0 + 1, :]
                    dst = bass.AP(tensor=dvw.tensor, offset=dvw.offset,
                                  ap=[[512, 100], [2 * C * 512, 4], [1, 512]])
                    nc.gpsimd.dma_start(out=dst, in_=src)

            # ---------------- pipelined main loop ----------------
            emit_load(0)
            emit_load(1)
            for i in range(4):
                emit_xt(0, i)

            for it in range(N_W):
                if it + 2 < N_W:
                    emit_load(it + 2)
                for o in range(4):
                    if it + 1 < N_W:
                        emit_xt(it + 1, o)
                    emit_mm1(it, o)
                    if it >= 2:
                        emit_step5(it - 2, o)
                    if o == 1 and it >= 1:
                        emit_et(it - 1)
                        emit_norm(it - 1)
                emit_mm2(it)
                emit_softmax(it)
                if it == N_W - 1:
                    emit_et(it)
                    emit_norm(it)

            # epilogue: drain with 1-bank psum pieces (ph tag is free)
            def emit_step5_drain(w, t):
                g = t
                attn = state[("attn", w)]
                ti = t // 2
                if t % 2 == 0:
                    state[("osb", w, ti)] = sb.tile(
                        [128, 4, 512], F16, tag="osb", bufs=4,
                        name=f"osbd{w}_{ti}")
                osb = state[("osb", w, ti)]
                for half in range(2):
                    po = ps.tile([128, 512], F32, tag="ph", bufs=3,
                                 name=f"pod{w}_{t}_{half}")
                    nc.tensor.matmul(
                        po[:100, :],
                        attn[64 * half:64 * half + 64, g, :],
                        xp_g(w, g, 64 * half, 64),
                        start=True, stop=True,
                        tile_position=(64 * half, 0),
                    )
                    dchunk = osb[:100, 2 * (t % 2) + half, :]
                    if half == 0:
                        nc.scalar.copy(dchunk, po[:100, :])
                    else:
                        nc.vector.tensor_copy(dchunk, po[:100, :])
                if t % 2 == 1:
                    osb = state.pop(("osb", w, ti))
                    src = osb[0:100, :, :]
                    row0 = (WAVE_S * w + 8 * ti) * C
                    dvw = o_d[row0:row0 + 1, :]
                    dst = bass.AP(tensor=dvw.tensor, offset=dvw.offset,
                                  ap=[[512, 100], [2 * C * 512, 4], [1, 512]])
                    nc.gpsimd.dma_start(out=dst, in_=src)

            for t in range(4):
                emit_step5_drain(N_W - 2, t)
            for t in range(4):
                emit_step5_drain(N_W - 1, t)

    nc.compile()
    return nc


def kernel(**inputs):
    global LAST_RESULT
    if "nc" not in _CACHE:
        _CACHE["nc"] = build_nc()
    nc = _CACHE["nc"]

    x = np.asarray(inputs["input_tensor"], dtype=np.float32).astype(np.float16)
    w = np.asarray(inputs["W"], dtype=np.float32).astype(np.float16)
    bb = np.ascontiguousarray(inputs["b"], dtype=np.float32)
    cv = np.asarray(inputs["context_vector"], dtype=np.float32).astype(np.float16)

    in_maps = [
        {
            "input_tensor": np.ascontiguousarray(
                x[ci * B_LOC:(ci + 1) * B_LOC]).reshape(TW, H),
            "W": w,
            "b": bb,
            "context_vector": cv,
        }
        for ci in range(N_CORES)
    ]
    res = run_bass_kernel_spmd(nc, in_maps, core_ids=list(range(N_CORES)))
    LAST_RESULT = res
    out = np.empty((B, S, C, H), dtype=np.float32)
    for ci in range(N_CORES):
        out[ci * B_LOC:(ci + 1) * B_LOC] = (
            res.results[ci]["out"].astype(np.float32)
            .reshape(B_LOC, S, C, H))
    return out


# revision 17
# speedup vs baseline: 1.0856x; 1.0024x over previous
"""AttentionPerLabelWordLevel Trainium2 kernel (8-core SPMD, batch-sharded).

Reference computation (per batch b):
  h = tanh(x @ W.T + b)                      # [T, H]
  logits = h @ C.T                           # [S, L, C]
  m = max_L(logits)                          # [S, 1, C]
  attn = softmax_C(logits - m)               # [S, L, C]
  out[s, c, :] = sum_l attn[s, l, c] * x[s, l, :]   # [S, C, H]

Shapes: B=32, T=2500 (S=100 sentences x L=25 words), H=512, C=50.
Sharding: data-parallel over batch, 4 batches per core, processed as one
stream of 400 sentences = 25 waves x 16 sentences (no partial waves).

Per-core pipeline (x, W, C pre-cast to f16 on host):
  - x DMA'd per wave into a packed tile [128, 4x520]: 4 groups of 4
    sentences at partition offsets 0/32/64/96 (32 rows per slot so pad
    rows hold real, finite data).
  - x^T via 16 f16 identity matmuls (M=128 -> fast weight load) into
    f32 PSUM, copied to f16 SBUF by DVE.
  - h^T = tanh(W @ x^T + b) per o-chunk; logits[c,t] accumulated over o.
  - Softmax: max over words (strided), subtract, exp -> e[c,t] f16 with
    pad columns zeroed on gpsimd.
  - e^T via 4 identity matmuls -> f32 PSUM; 4 strided DVE multiplies
    build a block-diagonal, softmax-normalized attn tile [128, 4x100]
    (normalization fused into the scatter as a broadcast multiply).
  - Output einsum: 8 matmuls per wave (2 sentences each: K=64 row-packed,
    M=100) into 2-bank f32 PSUM tiles; output DMA'd straight from PSUM
    (f32, 4 sentences per DMA, fully contiguous in DRAM) on the gpsimd
    SWDGE queue - no staging copies.
  - Software pipeline: e^T lags 1 wave, output einsum lags 2 waves and is
    sprinkled between mm1 chunks so the PE never waits on the softmax
    chain and PSUM-store drains overlap compute.
"""

import numpy as np

import concourse.bacc as bacc
import concourse.bass as bass
import concourse.tile as tile
from concourse import mybir
from concourse.bass_utils import run_bass_kernel_spmd
from concourse.masks import make_identity

F32 = mybir.dt.float32
F16 = mybir.dt.float16
AX = mybir.AxisListType
AF = mybir.ActivationFunctionType

N_CORES = 8
B = 32
S = 100          # sentences per batch
L = 25           # words per sentence
C = 50           # classes
H = 512          # hidden
B_LOC = B // N_CORES          # batches per core
NSENT = B_LOC * S             # 400 sentences per core
WAVE_S = 16                   # sentences per wave
N_W = NSENT // WAVE_S         # 25 full waves
TW = 10000                    # total words per core
WSTRIDE = 4 * L * H           # 4-sentence stride in x (elements)

_CACHE = {}
LAST_RESULT = None


def build_nc():
    nc = bacc.Bacc(trn_type="TRN2", target_bir_lowering=False, debug=False,
                   num_swdge_queues=2)
    x_d = nc.declare_dram_parameter("input_tensor", [TW, H], F16, isOutput=False)
    w_d = nc.declare_dram_parameter("W", [H, H], F16, isOutput=False)
    b_d = nc.declare_dram_parameter("b", [H], F32, isOutput=False)
    c_d = nc.declare_dram_parameter("context_vector", [C, H], F16, isOutput=False)
    o_d = nc.declare_dram_parameter("out", [NSENT * C, H], F16, isOutput=True)

    with tile.TileContext(nc) as tc:
        with tc.tile_pool(name="sb", bufs=1) as sb, \
             tc.tile_pool(name="consts", bufs=1) as consts, \
             tc.tile_pool(name="ps", bufs=1, space="PSUM") as ps:

            # ---------------- one-time setup ----------------
            ident_f = consts.tile([128, 128], F32)
            make_identity(nc, ident_f)
            ident_h = consts.tile([128, 128], F16)
            nc.vector.tensor_copy(ident_h, ident_f)

            # W^T tiles via PE transposes: w_t[i] is [i-part 128, o 512] f16
            w_nat = []
            for o in range(4):
                wn = consts.tile([128, 512], F16, name=f"w_nat{o}")
                nc.scalar.dma_start(out=wn, in_=w_d[o * 128:(o + 1) * 128, :])
                w_nat.append(wn)
            c_h = consts.tile([64, 512], F16)
            nc.gpsimd.dma_start(out=c_h[:C, :], in_=c_d[:, :])
            b_sb = consts.tile([128, 4], F32)
            nc.scalar.dma_start(out=b_sb, in_=b_d.rearrange("(k p) -> p k", p=128))

            # o-ordered so transposes start as soon as w_nat[0] lands
            pwt2 = [ps.tile([128, 1024], F16, tag="misc", bufs=3,
                            name=f"pwt2_{hh}") for hh in range(2)]
            for o in range(4):
                for i in range(4):
                    nc.tensor.transpose(
                        pwt2[i // 2][:, 512 * (i % 2) + o * 128:
                                     512 * (i % 2) + (o + 1) * 128],
                        w_nat[o][:, i * 128:(i + 1) * 128],
                        ident_h,
                    )
            w_t = []
            for i in range(4):
                wt = consts.tile([128, 512], F16, name=f"w_t{i}")
                nc.vector.tensor_copy(
                    wt.bitcast(F32),
                    pwt2[i // 2][:, 512 * (i % 2):512 * (i % 2 + 1)]
                    .bitcast(F32))
                w_t.append(wt)

            # C^T tile: [o-part 128, o_chunk 4, c 64] f16
            pct = ps.tile([128, 4, 64], F16, tag="misc", bufs=3, name="pct")
            for o in range(4):
                nc.tensor.transpose(
                    pct[:, o, :C], c_h[:C, o * 128:(o + 1) * 128],
                    ident_h[:C, :C],
                )
            c_t = consts.tile([128, 4, 64], F16)
            nc.vector.tensor_copy(c_t.bitcast(F32), pct.bitcast(F32))

            # ---------------- per-wave pieces ----------------
            state = {}

            def emit_load(w):
                """DMA x for wave w into a packed tile (4 DMAs on sync)."""
                xp_all = sb.tile([128, 2088], F16, tag="xp", bufs=5,
                                 name=f"xp{w}")
                nrows = 32 if w < N_W - 1 else L
                qs = ([nc.sync, nc.sync, nc.gpsimd, nc.gpsimd] if w < 2
                      else [nc.sync] * 4)
                for jj in range(4):
                    dvw = xp_all[32 * jj:32 * jj + nrows, :]
                    dst = bass.AP(tensor=xp_all.tensor, offset=dvw.offset,
                                  ap=[dvw.ap[0], [520, 4], [1, 512]])
                    w0 = (WAVE_S * w + jj) * L
                    svw = x_d[w0:w0 + 1, :]
                    src = bass.AP(tensor=svw.tensor, offset=svw.offset,
                                  ap=[[512, nrows], [WSTRIDE, 4], [1, 512]])
                    qs[jj].dma_start(out=dst, in_=src)
                state[("xp", w)] = xp_all

            def xp_g(w, g, r0, nr):
                xp_all = state[("xp", w)]
                return xp_all[r0:r0 + nr, 520 * g:520 * g + 512]

            def emit_xt(w, i):
                """x^T i-chunk via identity matmuls (f32 psum) + DVE copy."""
                if i == 0:
                    state[("xt", w)] = sb.tile([128, 2048], F16, tag="xt",
                                               bufs=2, name=f"xt{w}")
                xt = state[("xt", w)]
                pxt = ps.tile([128, 512], F32, tag="misc", bufs=3,
                              name=f"pxt{w}_{i}")
                for g in range(4):
                    nc.tensor.matmul(
                        pxt[:, 128 * g:128 * (g + 1)],
                        xp_g(w, g, 0, 128)[:, 128 * i:128 * (i + 1)],
                        ident_h,
                        start=True, stop=True,
                    )
                if i < 2:
                    nc.vector.tensor_copy(xt[:, 512 * i:512 * (i + 1)], pxt)
                else:
                    nc.scalar.copy(xt[:, 512 * i:512 * (i + 1)], pxt)

            def emit_mm1(w, o):
                """h^T[o] = tanh(W[o,:] @ x^T + b[o])."""
                if o == 0:
                    state[("h", w)] = sb.tile([128, 2048], F16, tag="h",
                                              bufs=2, name=f"h{w}")
                h = state[("h", w)]
                xt = state[("xt", w)]
                ph = ps.tile([128, 512], F32, tag="ph", bufs=3,
                             name=f"ph{w}_{o}")
                for i in range(4):
                    nc.tensor.matmul(
                        ph, w_t[i][:, o * 128:(o + 1) * 128],
                        xt[:, 512 * i:512 * (i + 1)],
                        start=(i == 0), stop=(i == 3),
                    )
                nc.scalar.activation(
                    out=h[:, 512 * o:512 * (o + 1)], in_=ph,
                    func=AF.Tanh, bias=b_sb[:, o:o + 1], scale=1.0,
                )

            def emit_mm2(w):
                """logits[c, t] accumulated over o-chunks."""
                h = state[("h", w)]
                pl = ps.tile([64, 512], F32, tag="misc", bufs=3,
                             name=f"pl{w}")
                for o in range(4):
                    nc.tensor.matmul(
                        pl[:C, :], c_t[:, o, :C], h[:, 512 * o:512 * (o + 1)],
                        start=(o == 0), stop=(o == 3),
                    )
                state[("pl", w)] = pl

            def emit_softmax(w):
                """m = max_L, e = exp(logits - m), pad cols of e zeroed."""
                pl = state.pop(("pl", w))
                m = sb.tile([64, 16], F32, tag="m", bufs=2, name=f"m{w}")
                epre = sb.tile([64, 512], F16, tag="epre", bufs=2,
                               name=f"epre{w}")
                e_sb = sb.tile([64, 512], F16, tag="e", bufs=2,
                               name=f"e{w}")
                pv = e_sb[:C, 25:]
                pad_v = bass.AP(tensor=e_sb.tensor, offset=pv.offset,
                                ap=[pv.ap[0], [32, 16], [1, 7]])
                nc.gpsimd.memset(pad_v, 0.0)
                plv = pl[:C, :]
                pl_v = bass.AP(tensor=pl.tensor, offset=plv.offset,
                               ap=[plv.ap[0], [32, 16], [1, L]])
                nc.vector.reduce_max(out=m[:C, :], in_=pl_v, axis=AX.X)
                epv = epre[:C, :]
                ep_v = bass.AP(tensor=epre.tensor, offset=epv.offset,
                               ap=[epv.ap[0], [32, 16], [1, L]])
                ev = e_sb[:C, :]
                e_v = bass.AP(tensor=e_sb.tensor, offset=ev.offset,
                              ap=[ev.ap[0], [32, 16], [1, L]])
                mv = m[:C, :]
                m_v = bass.AP(tensor=m.tensor, offset=mv.offset,
                              ap=[mv.ap[0], [1, 16], [0, L]])
                nc.vector.tensor_sub(ep_v, pl_v, m_v)
                nc.scalar.activation(out=e_v, in_=ep_v, func=AF.Exp)
                state[("e", w)] = e_sb

            def emit_et(w):
                """e^T via 4 identity matmuls -> f32 psum (pe4)."""
                e_sb = state.pop(("e", w))
                pe4 = ps.tile([128, 4, 64], F32, tag="misc", bufs=3,
                              name=f"pe4_{w}")
                for g in range(4):
                    nc.tensor.matmul(
                        pe4[:, g, :C],
                        e_sb[:C, 128 * g:128 * (g + 1)],
                        ident_h[:C, :C],
                        start=True, stop=True,
                    )
                state[("pe4", w)] = pe4

            def emit_norm(w):
                """Block-diagonal attn tile with fused softmax-C normalize."""
                pe4 = state.pop(("pe4", w))
                attn = sb.tile([128, 4, 100], F16, tag="attn", bufs=3,
                               name=f"attn{w}")
                nc.gpsimd.memset(attn, 0.0)
                z = sb.tile([128, 4], F32, tag="z", bufs=2, name=f"z{w}")
                sv = pe4[:, :, :C]
                sum_v = bass.AP(tensor=pe4.tensor, offset=sv.offset,
                                ap=[sv.ap[0], [64, 4], [1, C]])
                nc.vector.reduce_sum(out=z, in_=sum_v, axis=AX.X)
                nc.vector.tensor_scalar_max(z, z, 1e-30)
                nc.vector.reciprocal(out=z, in_=z)
                # scatter+normalize: row block rb -> cols 0-49 (rb even) or
                # 50-99 (rb odd); multiply by z broadcast along C
                for rb in range(4):
                    r0 = 32 * rb
                    co = 50 * (rb % 2)
                    sv = pe4[r0:r0 + 32, :, :C]
                    src = bass.AP(tensor=pe4.tensor, offset=sv.offset,
                                  ap=[sv.ap[0], [64, 4], [1, C]])
                    zv = z[r0:r0 + 32, :]
                    z_v = bass.AP(tensor=z.tensor, offset=zv.offset,
                                  ap=[zv.ap[0], [1, 4], [0, C]])
                    dv = attn[r0:r0 + 32, :, co:co + C]
                    dst = bass.AP(tensor=attn.tensor, offset=dv.offset,
                                  ap=[dv.ap[0], [100, 4], [1, C]])
                    nc.vector.tensor_mul(dst, src, z_v)
                state[("attn", w)] = attn

            def emit_step5(w, t):
                """Output pair t (t=0..3): 4 sentences = 2 matmuls into a
                2-bank f32 psum tile, one cast-copy to f16 staging, and a
                DMA per 8 sentences."""
                g = t
                attn = state[("attn", w)]
                po = ps.tile([128, 1024], F32, tag="po", bufs=1,
                             name=f"po{w}_{t}")
                for half in range(2):
                    nc.tensor.matmul(
                        po[:100, 512 * half:512 * (half + 1)],
                        attn[64 * half:64 * half + 64, g, :],
                        xp_g(w, g, 64 * half, 64),
                        start=True, stop=True,
                        tile_position=(64 * half, 0),
                    )
                ti = t // 2
                if t % 2 == 0:
                    state[("osb", w, ti)] = sb.tile(
                        [128, 4, 512], F16, tag="osb", bufs=4,
                        name=f"osb{w}_{ti}")
                osb = state[("osb", w, ti)]
                dchunk = osb[:100, 2 * (t % 2):2 * (t % 2) + 2, :]
                if t % 2 == 0:
                    nc.scalar.copy(dchunk, po[:100, :])
                else:
                    nc.vector.tensor_copy(dchunk, po[:100, :])
                if t % 2 == 1:
                    osb = state.pop(("osb", w, ti))
                    src = osb[0:100, :, :]
                    row0 = (WAVE_S * w + 8 * ti) * C
                    dvw = o_d[row0:row0 + 1, :]
                    dst = bass.AP(tensor=dvw.tensor, offset=dvw.offset,
                                  ap=[[512, 100], [2 * C * 512, 4], [1, 512]])
                    nc.gpsimd.dma_start(out=dst, in_=src)

            # ---------------- pipelined main loop ----------------
            emit_load(0)
            emit_load(1)
            for i in range(4):
                emit_xt(0, i)

            for it in range(N_W):
                if it + 2 < N_W:
                    emit_load(it + 2)
                for o in range(4):
                    if it + 1 < N_W:
                        emit_xt(it + 1, o)
                    emit_mm1(it, o)
                    if it >= 2:
                        emit_step5(it - 2, o)
                    if o == 1 and it >= 1:
                        emit_et(it - 1)
                        emit_norm(it - 1)
                emit_mm2(it)
                emit_softmax(it)
                if it == N_W - 1:
                    emit_et(it)
                    emit_norm(it)

            # epilogue: drain with 1-bank psum pieces (ph tag is free)
            def emit_step5_drain(w, t):
                g = t
                attn = state[("attn", w)]
                ti = t // 2
                if t % 2 == 0:
                    state[("osb", w, ti)] = sb.tile(
                        [128, 4, 512], F16, tag="osb", bufs=4,
                        name=f"osbd{w}_{ti}")
                osb = state[("osb", w, ti)]
                for half in range(2):
                    po = ps.tile([128, 512], F32, tag="ph", bufs=3,
                                 name=f"pod{w}_{t}_{half}")
                    nc.tensor.matmul(
                        po[:100, :],
                        attn[64 * half:64 * half + 64, g, :],
                        xp_g(w, g, 64 * half, 64),
                        start=True, stop=True,
                        tile_position=(64 * half, 0),
                    )
                    dchunk = osb[:100, 2 * (t % 2) + half, :]
                    if half == 0:
                        nc.scalar.copy(dchunk, po[:100, :])
                    else:
                        nc.vector.tensor_copy(dchunk, po[:100, :])
                src = osb[0:100, 2 * (t % 2):2 * (t % 2) + 2, :]
                row0 = (WAVE_S * w + 4 * t) * C
                dvw = o_d[row0:row0 + 1, :]
                dst = bass.AP(tensor=dvw.tensor, offset=dvw.offset,
                              ap=[[512, 100], [2 * C * 512, 2], [1, 512]])
                q = [nc.gpsimd, nc.sync, nc.scalar][(4 * w + t) % 3]
                q.dma_start(out=dst, in_=src)
                if t % 2 == 1:
                    state.pop(("osb", w, ti))

            for t in range(4):
                emit_step5_drain(N_W - 2, t)
            for t in range(4):
                emit_step5_drain(N_W - 1, t)

    nc.compile()
    return nc


def kernel(**inputs):
    global LAST_RESULT
    if "nc" not in _CACHE:
        _CACHE["nc"] = build_nc()
    nc = _CACHE["nc"]

    x = np.asarray(inputs["input_tensor"], dtype=np.float32).astype(np.float16)
    w = np.asarray(inputs["W"], dtype=np.float32).astype(np.float16)
    bb = np.ascontiguousarray(inputs["b"], dtype=np.float32)
    cv = np.asarray(inputs["context_vector"], dtype=np.float32).astype(np.float16)

    in_maps = [
        {
            "input_tensor": np.ascontiguousarray(
                x[ci * B_LOC:(ci + 1) * B_LOC]).reshape(TW, H),
            "W": w,
            "b": bb,
            "context_vector": cv,
        }
        for ci in range(N_CORES)
    ]
    res = run_bass_kernel_spmd(nc, in_maps, core_ids=list(range(N_CORES)))
    LAST_RESULT = res
    out = np.empty((B, S, C, H), dtype=np.float32)
    for ci in range(N_CORES):
        out[ci * B_LOC:(ci + 1) * B_LOC] = (
            res.results[ci]["out"].astype(np.float32)
            .reshape(B_LOC, S, C, H))
    return out
